# revision 1
# baseline (speedup 1.0000x reference)
"""Single-head causal attention (B=8, S=2048, D=1024, H=128) on 8 trn2 cores.

Data-parallel over batch (1 element per core). Per core:
  P1: Q^T/K^T/V^T projections. PREC=1 (default): 3-pass split-bf16
      matmuls (hi*hi + lo*hi + hi*lo) with x and W split on the host —
      ~fp32-grade accuracy at 3 bf16 passes (vs 4 for native fp32).
      Biases folded in as rank-1 (K=1) matmuls, also hi/lo split.
  P2: per 128-row strip: scores S = Q^T_strip.T @ K^T over the causal
      extent into PSUM (3-pass split-bf16 of the on-chip Q^T/K^T),
      causal mask added on the diagonal tile via a transpose-mode
      accumulate, row max on DVE, exp on ACT (scale=32 folded in,
      bias=-32*max, accum_out produces row sums), P stored bf16,
      P^T tiles via PE transpose + DVE copyback, PV accumulation with
      P^T stationary; finalize divides by row sums during PSUM->SBUF.

x^T is required (contraction dim must be on partitions for both matmul
operands), produced on the host during sharding.
"""
import os
import sys

sys.path.insert(0, "/opt/trn_rl_repo")
import numpy as np
import ml_dtypes

import concourse.bass as bass
import concourse.mybir as mybir
import concourse.tile as tile
from concourse import bacc
from concourse.bass_utils import run_bass_kernel_spmd
from concourse.masks import make_identity
from concourse.tile_rust import add_dep_helper

B, S, D, H = 8, 2048, 1024, 128
NK = D // 128          # 8 d-tiles
NS = S // 128          # 16 strips / t-tiles
CH = 512               # psum chunk width
NCH = S // CH          # 4 chunks across full seq

F32 = mybir.dt.float32
F32R = mybir.dt.float32r
BF16 = mybir.dt.bfloat16

PREC = os.environ.get("PREC", "1") == "1"      # split-bf16 (exact-ish) vs all-f32r
MASK_DVE = os.environ.get("MASK_DVE", "0") == "1"

_NC_CACHE = {}


def _build_precise():
    nc = bacc.Bacc()
    XMERGE = os.environ.get("XMERGE", "0") == "1"
    if XMERGE:
        xhl_d = nc.declare_dram_parameter("xhl", [2, D, S], BF16, isOutput=False)
    else:
        xh_d = nc.declare_dram_parameter("xh", [D, S], BF16, isOutput=False)
        xl_d = nc.declare_dram_parameter("xl", [D, S], BF16, isOutput=False)
    Wh_d = [
        nc.declare_dram_parameter(f"W{n}h", [D, H], BF16, isOutput=False)
        for n in "qkv"
    ]
    Wl_d = [
        nc.declare_dram_parameter(f"W{n}l", [D, H], BF16, isOutput=False)
        for n in "qkv"
    ]
    bh_d = [
        nc.declare_dram_parameter(f"b{n}h", [1, H], BF16, isOutput=False)
        for n in "qkv"
    ]
    bl_d = [
        nc.declare_dram_parameter(f"b{n}l", [1, H], BF16, isOutput=False)
        for n in "qkv"
    ]
    ones_d = nc.declare_dram_parameter("ones", [1, CH], BF16, isOutput=False)
    out_d = nc.declare_dram_parameter("outT", [H, S], F32, isOutput=True)
    sums_d = nc.declare_dram_parameter("sums", [128, NS], F32, isOutput=True)

    with tile.TileContext(nc) as tc:
        with (
            tc.tile_pool(name="cons", bufs=1) as cons,
            tc.tile_pool(name="qkv", bufs=1) as qkv,
            tc.tile_pool(name="pp", bufs=2) as pp,
            tc.tile_pool(name="outp", bufs=4) as outp,
            tc.tile_pool(name="stat", bufs=6) as stat,
        ):
            # ---- constants ----
            wh_sb = [cons.tile([128, NK, H], BF16, name=f"wh{p}", tag=f"wh{p}") for p in range(3)]
            wl_sb = [cons.tile([128, NK, H], BF16, name=f"wl{p}", tag=f"wl{p}") for p in range(3)]
            WQ0 = os.environ.get("WQ0", "0") == "1"
            if WQ0:
                nc.sync.dma_start(out=wh_sb[0][:, 0, :], in_=Wh_d[0][0:128, :])
                nc.sync.dma_start(
                    out=wh_sb[0][:, 1:NK, :],
                    in_=Wh_d[0][128:, :].rearrange("(k p) h -> p k h", p=128),
                )
            else:
                nc.sync.dma_start(out=wh_sb[0], in_=Wh_d[0].rearrange("(k p) h -> p k h", p=128))
            b2_sb = [cons.tile([2, H], BF16, name=f"b2{p}", tag=f"b2{p}") for p in range(3)]
            ones_sb = cons.tile([2, CH], BF16, tag="ones")

            identb = cons.tile([128, 128], BF16, tag="identb")
            make_identity(nc, identb)
            identf = cons.tile([128, 128], F32, tag="identf")
            make_identity(nc, identf)
            # maskT[t, s] = -1e30 where s < t; its PE transpose is the
            # additive causal mask for a diagonal score tile.
            maskT = cons.tile([128, 128], F32, tag="maskT")
            nc.gpsimd.memset(maskT, 0.0)
            nc.gpsimd.affine_select(
                out=maskT, in_=maskT, compare_op=mybir.AluOpType.is_ge,
                fill=-1e30, base=0, pattern=[[1, 128]], channel_multiplier=-1,
            )
            if MASK_DVE:
                mask_sb = cons.tile([128, 128], F32, tag="mask_sb")
                nc.gpsimd.memset(mask_sb, 0.0)
                nc.gpsimd.affine_select(
                    out=mask_sb, in_=mask_sb, compare_op=mybir.AluOpType.is_ge,
                    fill=-1e30, base=0, pattern=[[-1, 128]], channel_multiplier=1,
                )

            qth = qkv.tile([128, S], BF16, tag="qth")
            qtl = qkv.tile([128, S], BF16, tag="qtl")
            kth = qkv.tile([128, S], BF16, tag="kth")
            ktl = qkv.tile([128, S], BF16, tag="ktl")
            vt_bf = qkv.tile([128, S], BF16, tag="vt")
            VXBAR = os.environ.get("VXBAR", "0") == "1"
            v_sb = qkv.tile([128, NS, 160 if VXBAR else H], BF16, tag="v")
            sums_all = qkv.tile([128, NS], F32, tag="sums_all")

            with (
                tc.tile_pool(name="xtp", bufs=1) as xtp,
                tc.tile_pool(name="ps_a", bufs=int(os.environ.get("SCB", "6")), space="PSUM") as ps_a,
            ):
                # ---- P1: load x^T hi/lo, project ----
                if XMERGE:
                    xhl = [xtp.tile([128, 2, S], BF16, name=f"xhl{k}", tag=f"xhl{k}") for k in range(NK)]
                    xh = [t[:, 0, :] for t in xhl]
                    xl = [t[:, 1, :] for t in xhl]
                else:
                    xh = [xtp.tile([128, S], BF16, name=f"xh{k}", tag=f"xh{k}") for k in range(NK)]
                    xl = [xtp.tile([128, S], BF16, name=f"xl{k}", tag=f"xl{k}") for k in range(NK)]
                HD = S // 2
                XO = os.environ.get("XO", "old")
                if XMERGE:
                    for k in range(NK):
                        nc.sync.dma_start(
                            out=xhl[k][:, :, 0:HD],
                            in_=xhl_d[:, 128 * k : 128 * (k + 1), 0:HD].rearrange("g p s -> p g s"),
                        )
                        if k == 0:
                            for p in (1, 2):
                                nc.sync.dma_start(out=wh_sb[p], in_=Wh_d[p].rearrange("(k p) h -> p k h", p=128))
                            for p in range(3):
                                nc.sync.dma_start(out=wl_sb[p], in_=Wl_d[p].rearrange("(k p) h -> p k h", p=128))
                    for p in range(3):
                        nc.sync.dma_start(out=b2_sb[p][0:1, :], in_=bh_d[p][:, :])
                        nc.sync.dma_start(out=b2_sb[p][1:2, :], in_=bl_d[p][:, :])
                    nc.sync.dma_start(out=ones_sb[0:1, :], in_=ones_d[:, :])
                    nc.sync.dma_start(out=ones_sb[1:2, :], in_=ones_d[:, :])
                    for k in range(NK):
                        last_in_dma = nc.sync.dma_start(
                            out=xhl[k][:, :, HD:S],
                            in_=xhl_d[:, 128 * k : 128 * (k + 1), HD:S].rearrange("g p s -> p g s"),
                        )
                elif XO == "hfirst":
                    # all x-hi halves first: 5 of 7 projection passes per
                    # d-tile consume only hi inputs, so the PE ramps on half
                    # the stream; W-hi/lo slot in behind the first tile
                    for k in range(NK):
                        nc.sync.dma_start(out=xh[k][:, 0:HD], in_=xh_d[128 * k : 128 * (k + 1), 0:HD])
                        if k == 0:
                            for p in (1, 2):
                                nc.sync.dma_start(out=wh_sb[p], in_=Wh_d[p].rearrange("(k p) h -> p k h", p=128))
                            for p in range(3):
                                nc.sync.dma_start(out=wl_sb[p], in_=Wl_d[p].rearrange("(k p) h -> p k h", p=128))
                    for k in range(NK):
                        nc.sync.dma_start(out=xl[k][:, 0:HD], in_=xl_d[128 * k : 128 * (k + 1), 0:HD])
                    for k in range(NK):
                        nc.sync.dma_start(out=xh[k][:, HD:S], in_=xh_d[128 * k : 128 * (k + 1), HD:S])
                    for k in range(NK):
                        last_in_dma = nc.sync.dma_start(out=xl[k][:, HD:S], in_=xl_d[128 * k : 128 * (k + 1), HD:S])
                else:
                    WLAT = int(os.environ.get("WLAT", "0"))
                    XQ0 = os.environ.get("XQ0", "1") == "1"
                    for k in range(NK):
                        if k == 0 and XQ0:
                            nc.sync.dma_start(out=xh[k][:, 0:CH], in_=xh_d[0:128, 0:CH])
                            nc.sync.dma_start(out=xh[k][:, CH:HD], in_=xh_d[0:128, CH:HD])
                        else:
                            nc.sync.dma_start(out=xh[k][:, 0:HD], in_=xh_d[128 * k : 128 * (k + 1), 0:HD])
                        nc.sync.dma_start(out=xl[k][:, 0:HD], in_=xl_d[128 * k : 128 * (k + 1), 0:HD])
                        if k == 0:
                            for p in (1, 2):
                                nc.sync.dma_start(out=wh_sb[p], in_=Wh_d[p].rearrange("(k p) h -> p k h", p=128))
                        if k == WLAT:
                            # V is single-pass: its W-lo is never consumed, so
                            # don't spend ramp bandwidth loading it
                            for p in (0, 1):
                                nc.sync.dma_start(out=wl_sb[p], in_=Wl_d[p].rearrange("(k p) h -> p k h", p=128))
                    # small constant loads deferred out of the hot ramp
                    for p in range(3):
                        nc.sync.dma_start(out=b2_sb[p][0:1, :], in_=bh_d[p][:, :])
                        nc.sync.dma_start(out=b2_sb[p][1:2, :], in_=bl_d[p][:, :])
                    nc.sync.dma_start(out=ones_sb[0:1, :], in_=ones_d[:, :])
                    nc.sync.dma_start(out=ones_sb[1:2, :], in_=ones_d[:, :])
                    for k in range(NK):
                        nc.sync.dma_start(out=xh[k][:, HD:S], in_=xh_d[128 * k : 128 * (k + 1), HD:S])
                        last_in_dma = nc.sync.dma_start(out=xl[k][:, HD:S], in_=xl_d[128 * k : 128 * (k + 1), HD:S])

                ptb = pp.tile([128, NS, S], BF16, tag="pt", bufs=1)
                strip_p = {}
                TRDMA = os.environ.get("TRDMA", "0") == "1"
                dma_chain = [last_in_dma.ins if TRDMA else None]

                def emit_proj_groups(groups):
                    psums = {}
                    for (c, p) in groups:
                        psums[(c, p)] = ps_a.tile([128, CH], F32, name=f"pj{c}_{p}", tag="ps")
                    for k in range(NK):
                        for (c, p) in groups:
                            rhs_h = xh[k][:, CH * c : CH * (c + 1)]
                            nc.tensor.matmul(psums[(c, p)], wh_sb[p][:, k, :], rhs_h,
                                             start=(k == 0), stop=False)
                        # V is consumed in bf16; a single hi*hi pass is already
                        # at the bf16 noise floor, so the two correction passes
                        # run only for Q/K (emitted after all hi passes of this
                        # k so they don't stall on the later W-lo/x-lo DMAs)
                        for (c, p) in groups:
                            if p < 2:
                                rhs_h = xh[k][:, CH * c : CH * (c + 1)]
                                rhs_l = xl[k][:, CH * c : CH * (c + 1)]
                                nc.tensor.matmul(psums[(c, p)], wl_sb[p][:, k, :], rhs_h,
                                                 start=False, stop=False)
                                nc.tensor.matmul(psums[(c, p)], wh_sb[p][:, k, :], rhs_l,
                                                 start=False, stop=False)
                    for (c, p) in groups:
                        nc.tensor.matmul(psums[(c, p)], b2_sb[p], ones_sb, start=False, stop=True)
                        sl = slice(CH * c, CH * (c + 1))
                        if p < 2:
                            hi = (qth, kth)[p]
                            lo = (qtl, ktl)[p]
                            nc.scalar.activation(hi[:, sl], psums[(c, p)],
                                                 mybir.ActivationFunctionType.Copy)
                            nc.vector.tensor_sub(lo[:, sl], psums[(c, p)], hi[:, sl])
                        else:
                            nc.scalar.activation(vt_bf[:, sl], psums[(c, p)],
                                                 mybir.ActivationFunctionType.Copy)

                vdep = [None]

                def emit_vtransp_xbar():
                    tr_v = nc.sync.dma_start(out=v_sb[:, :, 0:128], in_=vt_bf, transpose=True)
                    add_dep_helper(tr_v.ins, last_in_dma.ins, sync=True,
                                   reason="serialize xbar vs x loads")
                    vdep[0] = tr_v.ins

                def emit_vtransp(j4):
                    vstage = ps_a.tile([128, 512], BF16, name=f"vst{j4}", tag="aux", bufs=int(os.environ.get("AUXB", "2")))
                    for m in range(4):
                        j = j4 + m
                        nc.tensor.matmul(vstage[:, 128 * m : 128 * (m + 1)],
                                         vt_bf[:, 128 * j : 128 * (j + 1)], identb,
                                         is_transpose=True, start=True, stop=True,
                                         skip_group_check=True)
                    nc.vector.tensor_copy(v_sb[:, j4 : j4 + 4, :], vstage)

                def emit_strip(i):
                    L = 128 * (i + 1)
                    qh = qth[:, 128 * i : 128 * (i + 1)]
                    ql = qtl[:, 128 * i : 128 * (i + 1)]
                    # diagonal 128-col chunk first (mask applies to it, off the
                    # critical path of the later chunk maxes), then 512-chunks
                    # covering [0, L-128)
                    if os.environ.get("DIAG1", "x") == "1":
                        spans = [(L - 128, L)]
                        c0 = 0
                        while c0 < L - 128:
                            spans.append((c0, min(c0 + CH, L - 128)))
                            c0 += CH
                    else:
                        spans = [(c0, min(c0 + CH, L)) for c0 in range(0, L, CH)]
                        if os.environ.get("DIAG1", "x") == "0r":
                            spans = spans[::-1]
                    scs = []
                    for (lo_, hi_) in spans:
                        w = hi_ - lo_
                        sc = ps_a.tile([128, CH], F32, name=f"sc{i}_{lo_}", tag="ps")
                        nc.tensor.matmul(sc[:, :w], qh, kth[:, lo_:hi_],
                                         start=True, stop=False)
                        nc.tensor.matmul(sc[:, :w], ql, kth[:, lo_:hi_],
                                         start=False, stop=False)
                        nc.tensor.matmul(sc[:, :w], qh, ktl[:, lo_:hi_],
                                         start=False, stop=(hi_ != L or MASK_DVE))
                        if hi_ == L:
                            if MASK_DVE:
                                nc.vector.tensor_tensor(
                                    out=sc[:, w - 128 : w], in0=sc[:, w - 128 : w],
                                    in1=mask_sb, op=mybir.AluOpType.add)
                            else:
                                nc.tensor.matmul(sc[:, w - 128 : w], maskT, identf,
                                                 is_transpose=True, start=False, stop=True,
                                                 skip_group_check=True)
                        scs.append((sc, lo_, w))
                    nch = len(scs)
                    # row max over the strip
                    st = stat.tile([128, 8], F32, tag="st")
                    for c, (sc, lo_, w) in enumerate(scs):
                        nc.vector.reduce_max(out=st[:, c : c + 1], in_=sc[:, :w],
                                             axis=mybir.AxisListType.X)
                    mxs = stat.tile([128, 1], F32, tag="mxs")
                    nc.vector.reduce_max(out=mxs, in_=st[:, :nch], axis=mybir.AxisListType.X)
                    nbias = stat.tile([128, 1], F32, tag="nbias")
                    nc.vector.tensor_scalar_mul(nbias, mxs, -32.0)
                    # exp (+ row sums) -> P bf16
                    p_sb = pp.tile([128, S], BF16, tag="p", bufs=int(os.environ.get("PBUF", "3")))
                    strip_p[i] = p_sb
                    sm = stat.tile([128, 8], F32, tag="sm")
                    for c, (sc, lo_, w) in enumerate(scs):
                        nc.scalar.activation(
                            p_sb[:, lo_ : lo_ + w], sc[:, :w],
                            mybir.ActivationFunctionType.Exp,
                            bias=nbias, scale=32.0, accum_out=sm[:, c : c + 1])
                    nc.vector.reduce_sum(out=sums_all[:, i : i + 1], in_=sm[:, :nch],
                                         axis=mybir.AxisListType.X)
                def emit_strip_pt(i):
                    p_sb = strip_p[i]
                    # P^T tiles
                    if TRDMA:
                        tr = nc.sync.dma_start(
                            out=ptb[:, 0 : i + 1, 128 * i : 128 * (i + 1)],
                            in_=p_sb[:, 0 : 128 * (i + 1)], transpose=True)
                        if dma_chain[0] is not None:
                            add_dep_helper(tr.ins, dma_chain[0], sync=True,
                                           reason="serialize xbar vs other DMA")
                        dma_chain[0] = tr.ins
                        return
                    for j4 in range(0, i + 1, 4):
                        jn = min(4, i + 1 - j4)
                        tstage = ps_a.tile([128, 512], BF16, name=f"tst{i}_{j4}", tag="aux", bufs=int(os.environ.get("AUXB", "2")))
                        for m in range(jn):
                            j = j4 + m
                            nc.tensor.matmul(tstage[:, 128 * m : 128 * (m + 1)],
                                             p_sb[:, 128 * j : 128 * (j + 1)], identb,
                                             is_transpose=True, start=True, stop=True,
                                             skip_group_check=True)
                        dst = ptb[:, j4 : j4 + jn, 128 * i : 128 * (i + 1)]
                        srcv = tstage[:, : 128 * jn].rearrange("p (a b) -> p a b", b=128)
                        if os.environ.get("CBSPLIT", "0") == "1" and (j4 // 4) % 2 == 1:
                            nc.scalar.activation(dst, srcv, mybir.ActivationFunctionType.Copy)
                        else:
                            nc.vector.tensor_copy(dst, srcv)

                def emit_band(gi):
                    # PV band over strips [4*gi .. 4*gi+3], then finalize them
                    b_lo = 512 * gi
                    b_hi = 512 * (gi + 1)
                    njs = 4 * gi + 4
                    oT = ps_a.tile([128, CH], F32, name=f"oT{gi}", tag=os.environ.get("OTT", "aux"), bufs=int(os.environ.get("OTB", "2")))
                    for j in range(njs):
                        lo = max(128 * j, b_lo) - b_lo
                        nc.tensor.matmul(oT[:, lo:], v_sb[:, j, 0:128],
                                         ptb[:, j, b_lo + lo : b_hi],
                                         start=(j == 0), stop=(j == njs - 1),
                                         skip_group_check=True)
                    osb = outp.tile([128, CH], F32, name=f"osb{gi}", tag="osb")
                    nc.vector.tensor_copy(osb, oT)
                    od = nc.sync.dma_start(out=out_d[:, b_lo:b_hi], in_=osb)
                    if vdep[0] is not None:
                        add_dep_helper(od.ins, vdep[0], sync=True,
                                       reason="serialize out stores vs xbar")
                    if TRDMA and dma_chain[0] is not None:
                        add_dep_helper(od.ins, dma_chain[0], sync=True,
                                       reason="serialize xbar vs other DMA")
                        dma_chain[0] = od.ins
                    if gi == 3:
                        nc.sync.dma_start(out=sums_d[:, :], in_=sums_all)

                ILV = os.environ.get("ILV", "0") == "1"
                if True:
                    ab = os.environ.get("AB", "2")
                    if ab == "2":
                        # full chunk-pair 0 (V included: it consumes the same
                        # early x tiles, giving the PE more work per arriving
                        # tile during the DMA-bound ramp), then early strips
                        emit_proj_groups([(c, p) for c in (0, 1) for p in range(3)])
                        for i in (0, 1, 2, 3):
                            emit_strip(i)
                        VTE = os.environ.get("VTE", "1") == "1"
                        if VTE:
                            # V tiles 0-7 depend only on chunk-pair 0's V
                            emit_vtransp(0)
                            emit_vtransp(4)
                        emit_proj_groups([(c, p) for c in (2, 3) for p in (0, 1)])
                        emit_proj_groups([(c, 2) for c in (2, 3)])
                    elif ab == "1":
                        # Q/K groups of chunks 0-1 first so early strips can
                        # start; everything else emitted behind them as
                        # PE gap-filler during softmax stalls
                        emit_proj_groups([(c, p) for c in (0, 1) for p in (0, 1)])
                        for i in (0, 1, 2, 3):
                            emit_strip(i)
                        emit_proj_groups([(c, 2) for c in (0, 1)])
                        emit_proj_groups([(c, p) for c in (2, 3) for p in (0, 1)])
                        emit_proj_groups([(c, 2) for c in (2, 3)])
                    if ab in ("1", "2"):
                        if os.environ.get("PTE", "0") == "1":
                            for i in (0, 1, 2, 3):
                                emit_strip_pt(i)
                        if VXBAR:
                            emit_vtransp_xbar()
                        else:
                            j4s = (8, 12) if (ab == "2" and os.environ.get("VTE", "1") == "1") else (0, 4, 8, 12)
                            for j4 in j4s:
                                emit_vtransp(j4)
                        if os.environ.get("PTDEF", "1") == "2":
                            # per-band batch: all softmaxes, then all P^T
                            for i in (0, 1, 2, 3):
                                emit_strip_pt(i)
                            emit_band(0)
                            for g in (1, 2, 3):
                                for i in range(4 * g, 4 * g + 4):
                                    emit_strip(i)
                                for i in range(4 * g, 4 * g + 4):
                                    emit_strip_pt(i)
                                emit_band(g)
                        elif os.environ.get("PTDEF", "1") == "1":
                            # P^T emission deferred one strip so the next
                            # strip's maxes outrank copybacks on DVE
                            BDEF = os.environ.get("BDEF", "0") == "1"
                            if os.environ.get("PTE", "0") != "1":
                                for i in (0, 1, 2, 3):
                                    emit_strip_pt(i)
                            if not BDEF:
                                emit_band(0)
                            for g in (1, 2, 3):
                                prev = None
                                for idx, i in enumerate(range(4 * g, 4 * g + 4)):
                                    emit_strip(i)
                                    if prev is not None:
                                        emit_strip_pt(prev)
                                    prev = i
                                    if BDEF and idx == 1:
                                        emit_band(g - 1)
                                emit_strip_pt(prev)
                                if not BDEF:
                                    emit_band(g)
                            if BDEF:
                                emit_band(3)
                        else:
                            for i in (0, 1, 2, 3):
                                emit_strip_pt(i)
                            emit_band(0)
                            for g in (1, 2, 3):
                                for i in range(4 * g, 4 * g + 4):
                                    emit_strip(i)
                                    emit_strip_pt(i)
                                emit_band(g)
                    if ab == "0":
                        emit_proj_groups([(c, p) for c in (0, 1) for p in range(3)])
                        emit_proj_groups([(c, p) for c in (2, 3) for p in range(3)])
                        if VXBAR:
                            emit_vtransp_xbar()
                        else:
                            for j4 in (0, 4, 8, 12):
                                emit_vtransp(j4)
                        done = set()
                        for i in range(NS):
                            emit_strip(i)
                            done.add(i)
                            g = i // 4
                            if all((4 * g + m) in done for m in range(4)):
                                emit_band(g)

    nc.compile()
    return nc


def _build_fast():
    nc = bacc.Bacc()
    xT_d = nc.declare_dram_parameter("xT", [D, S], F32R, isOutput=False)
    W_d = [nc.declare_dram_parameter(f"W{n}", [D, H], F32R, isOutput=False) for n in "qkv"]
    b_d = [nc.declare_dram_parameter(f"b{n}", [1, H], F32R, isOutput=False) for n in "qkv"]
    ones_d = nc.declare_dram_parameter("ones", [1, CH], F32R, isOutput=False)
    out_d = nc.declare_dram_parameter("outT", [H, S], F32, isOutput=True)
    sums_d = nc.declare_dram_parameter("sums", [128, NS], F32, isOutput=True)

    with tile.TileContext(nc) as tc:
        with (
            tc.tile_pool(name="xtp", bufs=1) as xtp,
            tc.tile_pool(name="cons", bufs=1) as cons,
            tc.tile_pool(name="qkv", bufs=1) as qkv,
            tc.tile_pool(name="pp", bufs=2) as pp,
            tc.tile_pool(name="outp", bufs=4) as outp,
            tc.tile_pool(name="stat", bufs=6) as stat,
            tc.tile_pool(name="ps512", bufs=6, space="PSUM") as ps512,
            tc.tile_pool(name="ps128", bufs=2, space="PSUM") as ps128,
        ):
            w_sb = [cons.tile([128, NK, H], F32R, name=f"w{p}", tag=f"w{p}") for p in range(3)]
            for p in range(3):
                nc.sync.dma_start(out=w_sb[p], in_=W_d[p].rearrange("(k p) h -> p k h", p=128))
            b_sb = [cons.tile([1, H], F32R, name=f"b{p}", tag=f"b{p}") for p in range(3)]
            for p in range(3):
                nc.sync.dma_start(out=b_sb[p], in_=b_d[p][:, :])
            ones_sb = cons.tile([1, CH], F32R, tag="ones")
            nc.sync.dma_start(out=ones_sb, in_=ones_d[:, :])

            identb = cons.tile([128, 128], BF16, tag="identb")
            make_identity(nc, identb)
            identf = cons.tile([128, 128], F32, tag="identf")
            make_identity(nc, identf)
            maskT = cons.tile([128, 128], F32, tag="maskT")
            nc.gpsimd.memset(maskT, 0.0)
            nc.gpsimd.affine_select(
                out=maskT, in_=maskT, compare_op=mybir.AluOpType.is_ge,
                fill=-1e30, base=0, pattern=[[1, 128]], channel_multiplier=-1)

            xt = [xtp.tile([128, S], F32R, name=f"xt{k}", tag=f"xt{k}") for k in range(NK)]
            for k in range(NK):
                nc.sync.dma_start(out=xt[k], in_=xT_d[128 * k : 128 * (k + 1), :])

            qt_sb = qkv.tile([128, S], F32R, tag="qt")
            kt_sb = qkv.tile([128, S], F32R, tag="kt")
            vt_bf = qkv.tile([128, S], BF16, tag="vt")
            VXBAR = os.environ.get("VXBAR", "0") == "1"
            v_sb = qkv.tile([128, NS, 160 if VXBAR else H], BF16, tag="v")

            for cpair in range(NCH // 2):
                groups = [(c, p) for c in (2 * cpair, 2 * cpair + 1) for p in range(3)]
                psums = {}
                for (c, p) in groups:
                    psums[(c, p)] = ps512.tile([128, CH], F32, name=f"pj{c}_{p}", tag="ps512")
                for k in range(NK):
                    for (c, p) in groups:
                        nc.tensor.matmul(psums[(c, p)], w_sb[p][:, k, :],
                                         xt[k][:, CH * c : CH * (c + 1)],
                                         start=(k == 0), stop=False)
                for (c, p) in groups:
                    nc.tensor.matmul(psums[(c, p)], b_sb[p], ones_sb, start=False, stop=True)
                    dst = (qt_sb, kt_sb, vt_bf)[p]
                    nc.scalar.activation(dst[:, CH * c : CH * (c + 1)], psums[(c, p)],
                                         mybir.ActivationFunctionType.Copy)

            for j in range(NS):
                vps = ps128.tile([128, 128], BF16, name=f"vps{j}", tag="ov")
                nc.tensor.matmul(vps, vt_bf[:, 128 * j : 128 * (j + 1)], identb,
                                 is_transpose=True, start=True, stop=True)
                nc.vector.tensor_copy(v_sb[:, j, :], vps)

            for i in range(NS):
                L = 128 * (i + 1)
                nch = (L + CH - 1) // CH
                scs = []
                for c in range(nch):
                    w = min(CH, L - CH * c)
                    sc = ps512.tile([128, CH], F32, name=f"sc{i}_{c}", tag="ps512")
                    nc.tensor.matmul(sc[:, :w], qt_sb[:, 128 * i : 128 * (i + 1)],
                                     kt_sb[:, CH * c : CH * c + w], start=True, stop=True)
                    scs.append((sc, w))
                sc_l, w_l = scs[-1]
                nc.tensor.matmul(sc_l[:, w_l - 128 : w_l], maskT, identf,
                                 is_transpose=True, start=False, stop=True,
                                 skip_group_check=True)
                st = stat.tile([128, 8], F32, tag="st")
                for c, (sc, w) in enumerate(scs):
                    nc.vector.reduce_max(out=st[:, c : c + 1], in_=sc[:, :w],
                                         axis=mybir.AxisListType.X)
                mxs = stat.tile([128, 1], F32, tag="mxs")
                nc.vector.reduce_max(out=mxs, in_=st[:, :nch], axis=mybir.AxisListType.X)
                nbias = stat.tile([128, 1], F32, tag="nbias")
                nc.vector.tensor_scalar_mul(nbias, mxs, -32.0)
                p_sb = pp.tile([128, S], BF16, tag="p", bufs=int(os.environ.get("PBUF", "3")))
                sm = stat.tile([128, 8], F32, tag="sm")
                for c, (sc, w) in enumerate(scs):
                    nc.scalar.activation(p_sb[:, CH * c : CH * c + w], sc[:, :w],
                                         mybir.ActivationFunctionType.Exp,
                                         bias=nbias, scale=32.0, accum_out=sm[:, c : c + 1])
                sums = stat.tile([128, 1], F32, tag="sums")
                nc.vector.reduce_sum(out=sums, in_=sm[:, :nch], axis=mybir.AxisListType.X)
                rec = stat.tile([128, 1], F32, tag="rec")
                nc.vector.reciprocal(rec, sums)
                ptb = pp.tile([128, NS, 128], BF16, tag="pt")
                for j in range(i + 1):
                    tps = ps512.tile([128, 128], BF16, name=f"tps{i}_{j}", tag="ps512")
                    nc.tensor.matmul(tps, p_sb[:, 128 * j : 128 * (j + 1)], identb,
                                     is_transpose=True, start=True, stop=True)
                    nc.vector.tensor_copy(ptb[:, j, :], tps)
                ov = ps128.tile([128, 128], F32, name=f"ov{i}", tag="ov")
                for j in range(i + 1):
                    nc.tensor.matmul(ov, ptb[:, j, :], v_sb[:, j, :],
                                     start=(j == 0), stop=(j == i))
                out_sb = outp.tile([128, H], F32, tag="osb")
                nc.vector.tensor_scalar_mul(out_sb, ov, rec)
                nc.sync.dma_start(out=out_d[128 * i : 128 * (i + 1), :], in_=out_sb)

    nc.compile()
    return nc


def _get_nc():
    key = (PREC, MASK_DVE, os.environ.get('SCB', '6'), os.environ.get('PBUF', '3'), os.environ.get('DESC', '0'), os.environ.get('ILV', '0'), os.environ.get('DIAG1', 'x'), os.environ.get('SORD', 'a'), os.environ.get('TRDMA', '0'), os.environ.get('VXBAR', '0'), os.environ.get('AB', '2'), os.environ.get('XO', 'old'), os.environ.get('WLAT', '0'), os.environ.get('WQ0', '0'), os.environ.get('XQ0', '1'), os.environ.get('CBSPLIT', '0'), os.environ.get('PTDEF', '1'), os.environ.get('BDEF', '0'), os.environ.get('XMERGE', '0'), os.environ.get('PTE', '0'), os.environ.get('VTE', '1'), os.environ.get('OTT', 'aux'), os.environ.get('OTB', '2'), os.environ.get('AUXB', '2'))
    if key not in _NC_CACHE:
        _NC_CACHE[key] = _build_precise() if PREC else _build_fast()
    return _NC_CACHE[key]


def _split_bf16(a):
    hi = a.astype(ml_dtypes.bfloat16)
    lo = (a - hi.astype(np.float32)).astype(ml_dtypes.bfloat16)
    return hi, lo


def make_in_maps(x, Wq, bq, Wk, bk, Wv, bv):
    x = np.asarray(x, np.float32)
    xt = np.ascontiguousarray(x.transpose(0, 2, 1))  # [B, D, S]
    in_maps = []
    if PREC:
        xth, xtl = _split_bf16(xt)
        xhl = np.stack([xth, xtl], axis=1)  # [B, 2, D, S]
        Ws = [_split_bf16(np.asarray(w, np.float32)) for w in (Wq, Wk, Wv)]
        bs = [_split_bf16(np.asarray(b, np.float32).reshape(1, H)) for b in (bq, bk, bv)]
        ones = np.ones((1, CH), ml_dtypes.bfloat16)
        for bi in range(B):
            if os.environ.get("XMERGE", "0") == "1":
                m = {"xhl": np.ascontiguousarray(xhl[bi]), "ones": ones}
            else:
                m = {"xh": np.ascontiguousarray(xth[bi]),
                     "xl": np.ascontiguousarray(xtl[bi]), "ones": ones}
            for p, n in enumerate("qkv"):
                m[f"W{n}h"], m[f"W{n}l"] = Ws[p]
                m[f"b{n}h"], m[f"b{n}l"] = bs[p]
            in_maps.append(m)
    else:
        ones = np.ones((1, CH), np.float32)
        for bi in range(B):
            m = {"xT": xt[bi], "ones": ones,
                 "Wq": np.asarray(Wq, np.float32), "Wk": np.asarray(Wk, np.float32),
                 "Wv": np.asarray(Wv, np.float32),
                 "bq": np.asarray(bq, np.float32).reshape(1, H),
                 "bk": np.asarray(bk, np.float32).reshape(1, H),
                 "bv": np.asarray(bv, np.float32).reshape(1, H)}
            in_maps.append(m)
    return in_maps


def kernel(x, Wq, bq, Wk, bk, Wv, bv):
    nc = _get_nc()
    in_maps = make_in_maps(x, Wq, bq, Wk, bk, Wv, bv)
    res = run_bass_kernel_spmd(nc, in_maps, list(range(B)))
    if PREC:
        outs = []
        for b in range(B):
            oT = res.results[b]["outT"]            # [H, S]
            sums = res.results[b]["sums"]          # [128, NS], s = 128*i + p
            s_flat = sums.T.reshape(S)
            outs.append((oT / s_flat[None, :]).T)
        return np.stack(outs).astype(np.float32)
    return np.stack([res.results[b]["out"] for b in range(B)]).astype(np.float32)



# revision 4
# speedup vs baseline: 1.3620x; 1.3620x over previous
"""Single-head causal attention (B=8, S=2048, D=1024, H=128) on 8 trn2 cores.

Data-parallel over batch (1 element per core). Per core, all matmuls run
single-pass in f32r (1 cycle/row on the PE for >=256-wide outputs, exact
fp32 numerics in this stack):

  P1: Q^T/K^T/V^T projections from x^T [D,S] f32r. The softmax scale
      (sqrt(D)=32) is folded into Q during the PSUM->SBUF copy on ACT
      (scale=32), and the biases ride the same copies (bias=[H,1] AP,
      bq pre-scaled by 32 on the host). V is stored bf16.
  P2: per 128-row strip i: scores = Q_strip.T @ K over the causal extent
      (spans chosen >=256 wide so f32r stays at 1 cyc/row), causal mask
      added on the diagonal span via a bf16 transpose-matmul, row max ->
      negated directly into the exp bias (reduce_max(negate=True)), exp
      on ACT writes P bf16 with accum_out producing per-chunk row sums
      (summed on the host), P^T via PE transpose + DVE copyback (or DMA
      xbar), PV accumulated per 512-wide band with V^T-transposed tiles.
      Output is written as out^T [H,S]; the host divides by the row sums.
"""
import os
import sys

sys.path.insert(0, "/opt/trn_rl_repo")
import numpy as np
import ml_dtypes

import concourse.bass as bass
import concourse.mybir as mybir
import concourse.tile as tile
from concourse import bacc
from concourse.bass_utils import run_bass_kernel_spmd
from concourse.masks import make_identity

B, S, D, H = 8, 2048, 1024, 128
NK = D // 128          # 8 d-tiles
NS = S // 128          # 16 strips
CH = 512               # psum chunk width
NCH = S // CH

F32 = mybir.dt.float32
F32R = mybir.dt.float32r
BF16 = mybir.dt.bfloat16

_NC_CACHE = {}


def _env(name, default):
    return os.environ.get(name, default)


def _spans_for(L):
    """Non-overlapping spans covering [0, L], all >=256 wide when possible
    (f32r matmul runs 4 cyc/row below 256), diagonal span last."""
    if L <= CH:
        return [(0, L)]
    rem = L % CH
    spans = []
    if rem == 0:
        first = CH
    elif rem == 128:
        first = 384  # last span will be 256
    else:
        first = rem  # 256 or 384
    spans.append((0, first))
    c0 = first
    while c0 < L:
        spans.append((c0, min(c0 + CH, L)))
        c0 += CH
    return spans


def _build():
    CBP = _env("CBP", "0") == "1"      # alternate P^T copybacks DVE/Pool
    PTX = int(_env("PTX", "99"))       # strips with i >= PTX use DMA xbar for P^T
    VX = _env("VX", "0") == "1"        # V transpose via DMA xbar
    PTW = int(_env("PTW", "1024"))     # ptb rolling window width (s cols)
    SCB = int(_env("SCB", "6"))
    PBUF = int(_env("PBUF", "3"))
    AUXB = int(_env("AUXB", "2"))
    OTB = int(_env("OTB", "2"))
    BDEF = _env("BDEF", "0") == "1"    # defer band PV one strip-group
    CBS = _env("CBS", "0") == "1"      # alternate P^T copybacks DVE/ACT

    nc = bacc.Bacc()
    xT_d = nc.declare_dram_parameter("xT", [D, S], F32R, isOutput=False)
    W_d = [nc.declare_dram_parameter(f"W{n}", [D, H], F32R, isOutput=False) for n in "qkv"]
    b_d = [nc.declare_dram_parameter(f"b{n}", [H, 1], F32, isOutput=False) for n in "qkv"]
    out_d = nc.declare_dram_parameter("outT", [H, S], F32, isOutput=True)
    sums_d = nc.declare_dram_parameter("sums", [128, NS * 4], F32, isOutput=True)

    with tile.TileContext(nc) as tc:
        with (
            tc.tile_pool(name="cons", bufs=1) as cons,
            tc.tile_pool(name="qkv", bufs=1) as qkv,
            tc.tile_pool(name="pp", bufs=2) as pp,
            tc.tile_pool(name="outp", bufs=4) as outp,
            tc.tile_pool(name="stat", bufs=8) as stat,
        ):
            # ---- constants ----
            w_sb = [cons.tile([128, NK, H], F32R, name=f"w{p}", tag=f"w{p}") for p in range(3)]
            b_sb = [cons.tile([128, 1], F32, name=f"b{p}", tag=f"b{p}") for p in range(3)]
            identb = cons.tile([128, 128], BF16, tag="identb")
            make_identity(nc, identb)
            identf = cons.tile([128, 128], F32, tag="identf")
            make_identity(nc, identf)
            # maskT[t, s] = -1e30 where s < t; its PE transpose is the
            # additive causal mask for the diagonal score tile.
            maskT = cons.tile([128, 128], F32, tag="maskT")
            nc.gpsimd.memset(maskT, 0.0)
            nc.gpsimd.affine_select(
                out=maskT, in_=maskT, compare_op=mybir.AluOpType.is_ge,
                fill=-1e30, base=0, pattern=[[1, 128]], channel_multiplier=-1,
            )

            qt = qkv.tile([128, S], F32R, tag="qt")
            kt = qkv.tile([128, S], F32R, tag="kt")
            vt_bf = qkv.tile([128, S], BF16, tag="vt")
            v_sb = qkv.tile([128, NS, H], BF16, tag="v")
            sums_sb = qkv.tile([128, NS * 4], F32, tag="sums")
            nc.gpsimd.memset(sums_sb, 0.0)

            with (
                tc.tile_pool(name="xtp", bufs=1) as xtp,
                tc.tile_pool(name="ps_a", bufs=SCB, space="PSUM") as ps_a,
            ):
                xt = [xtp.tile([128, S], F32R, name=f"xt{k}", tag=f"xt{k}") for k in range(NK)]
                HD = S // 2
                # ---- input DMA stream; first-needed first ----
                for p in range(3):
                    nc.sync.dma_start(out=w_sb[p][:, 0, :], in_=W_d[p][0:128, :])
                nc.sync.dma_start(out=xt[0][:, 0:CH], in_=xT_d[0:128, 0:CH])
                nc.sync.dma_start(out=xt[0][:, CH:HD], in_=xT_d[0:128, CH:HD])
                nc.sync.dma_start(out=xt[1][:, 0:HD], in_=xT_d[128:256, 0:HD])
                for p in range(3):
                    nc.sync.dma_start(
                        out=w_sb[p][:, 1:4, :],
                        in_=W_d[p][128:512, :].rearrange("(k p) h -> p k h", p=128),
                    )
                nc.sync.dma_start(out=xt[2][:, 0:HD], in_=xT_d[256:384, 0:HD])
                for p in range(3):
                    nc.sync.dma_start(
                        out=w_sb[p][:, 4:NK, :],
                        in_=W_d[p][512:1024, :].rearrange("(k p) h -> p k h", p=128),
                    )
                for k in range(3, NK):
                    nc.sync.dma_start(out=xt[k][:, 0:HD], in_=xT_d[128 * k : 128 * (k + 1), 0:HD])
                for p in range(3):
                    nc.sync.dma_start(out=b_sb[p], in_=b_d[p][:, :])
                for k in range(NK):
                    nc.sync.dma_start(out=xt[k][:, HD:S], in_=xT_d[128 * k : 128 * (k + 1), HD:S])

                ptb = pp.tile([128, NS, PTW], BF16, tag="pt", bufs=1)
                strip_p = {}

                def wcol(i):
                    return 128 * ((128 * i) % PTW // 128)

                def emit_proj(chunks):
                    psums = {}
                    for c in chunks:
                        for p in range(3):
                            psums[(c, p)] = ps_a.tile([128, CH], F32, name=f"pj{c}_{p}", tag="ps")
                    for k in range(NK):
                        for c in chunks:
                            for p in range(3):
                                nc.tensor.matmul(
                                    psums[(c, p)], w_sb[p][:, k, :],
                                    xt[k][:, CH * c : CH * (c + 1)],
                                    start=(k == 0), stop=(k == NK - 1),
                                )
                    for c in chunks:
                        sl = slice(CH * c, CH * (c + 1))
                        nc.scalar.activation(qt[:, sl], psums[(c, 0)],
                                             mybir.ActivationFunctionType.Identity,
                                             bias=b_sb[0], scale=32.0)
                        nc.scalar.activation(kt[:, sl], psums[(c, 1)],
                                             mybir.ActivationFunctionType.Identity,
                                             bias=b_sb[1], scale=1.0)
                        nc.scalar.activation(vt_bf[:, sl], psums[(c, 2)],
                                             mybir.ActivationFunctionType.Identity,
                                             bias=b_sb[2], scale=1.0)

                def emit_vtransp(j4):
                    vstage = ps_a.tile([128, 512], BF16, name=f"vst{j4}", tag="aux", bufs=AUXB)
                    for m in range(4):
                        j = j4 + m
                        nc.tensor.matmul(vstage[:, 128 * m : 128 * (m + 1)],
                                         vt_bf[:, 128 * j : 128 * (j + 1)], identb,
                                         is_transpose=True, start=True, stop=True,
                                         skip_group_check=True)
                    nc.vector.tensor_copy(v_sb[:, j4 : j4 + 4, :], vstage)

                def emit_vtransp_xbar():
                    nc.sync.dma_start(out=v_sb, in_=vt_bf, transpose=True)

                def emit_strip(i):
                    L = 128 * (i + 1)
                    qh = qt[:, 128 * i : 128 * (i + 1)]
                    spans = _spans_for(L)
                    scs = []
                    for (lo, hi) in spans:
                        w = hi - lo
                        sc = ps_a.tile([128, CH], F32, name=f"sc{i}_{lo}", tag="ps")
                        nc.tensor.matmul(sc[:, :w], qh, kt[:, lo:hi],
                                         start=True, stop=(hi != L))
                        if hi == L:
                            nc.tensor.matmul(sc[:, w - 128 : w], maskT, identf,
                                             is_transpose=True, start=False, stop=True,
                                             skip_group_check=True)
                        scs.append((sc, lo, w))
                    nch = len(scs)
                    nbias = stat.tile([128, 1], F32, tag="nbias")
                    if nch == 1:
                        nc.vector.reduce_max(out=nbias, in_=scs[0][0][:, : scs[0][2]],
                                             axis=mybir.AxisListType.X, negate=True)
                    else:
                        st = stat.tile([128, 4], F32, tag="st")
                        for c, (sc, lo, w) in enumerate(scs):
                            nc.vector.reduce_max(out=st[:, c : c + 1], in_=sc[:, :w],
                                                 axis=mybir.AxisListType.X)
                        nc.vector.reduce_max(out=nbias, in_=st[:, :nch],
                                             axis=mybir.AxisListType.X, negate=True)
                    p_sb = pp.tile([128, S], BF16, tag="p", bufs=PBUF)
                    strip_p[i] = p_sb
                    for c, (sc, lo, w) in enumerate(scs):
                        nc.scalar.activation(
                            p_sb[:, lo : lo + w], sc[:, :w],
                            mybir.ActivationFunctionType.Exp,
                            bias=nbias, scale=1.0,
                            accum_out=sums_sb[:, 4 * i + c : 4 * i + c + 1])

                def emit_strip_pt(i):
                    p_sb = strip_p[i]
                    wc = wcol(i)
                    if i >= PTX:
                        nc.sync.dma_start(
                            out=ptb[:, 0 : i + 1, wc : wc + 128],
                            in_=p_sb[:, 0 : 128 * (i + 1)], transpose=True)
                        return
                    for j4 in range(0, i + 1, 4):
                        jn = min(4, i + 1 - j4)
                        tstage = ps_a.tile([128, 512], BF16, name=f"tst{i}_{j4}",
                                           tag="aux", bufs=AUXB)
                        for m in range(jn):
                            j = j4 + m
                            nc.tensor.matmul(tstage[:, 128 * m : 128 * (m + 1)],
                                             p_sb[:, 128 * j : 128 * (j + 1)], identb,
                                             is_transpose=True, start=True, stop=True,
                                             skip_group_check=True)
                        dst = ptb[:, j4 : j4 + jn, wc : wc + 128]
                        srcv = tstage[:, : 128 * jn].rearrange("p (a b) -> p a b", b=128)
                        if CBS and (j4 // 4) % 2 == 1:
                            nc.scalar.activation(dst, srcv, mybir.ActivationFunctionType.Copy)
                        elif CBP and (j4 // 4) % 2 == 1:
                            nc.gpsimd.tensor_copy(dst, srcv)
                        else:
                            nc.vector.tensor_copy(dst, srcv)

                def emit_band(gi):
                    b_lo = 512 * gi
                    woff = b_lo % PTW
                    njs = 4 * gi + 4
                    oT = ps_a.tile([128, CH], F32, name=f"oT{gi}", tag="aux", bufs=OTB)
                    for j in range(njs):
                        lo = max(128 * j, b_lo) - b_lo
                        nc.tensor.matmul(oT[:, lo:], v_sb[:, j, :],
                                         ptb[:, j, woff + lo : woff + CH],
                                         start=(j == 0), stop=(j == njs - 1),
                                         skip_group_check=True)
                    osb = outp.tile([128, CH], F32, name=f"osb{gi}", tag="osb")
                    nc.vector.tensor_copy(osb, oT)
                    nc.sync.dma_start(out=out_d[:, b_lo : b_lo + CH], in_=osb)
                    if gi == 3:
                        nc.sync.dma_start(out=sums_d[:, :], in_=sums_sb)

                # ---- schedule ----
                emit_proj([0, 1])
                for i in (0, 1, 2, 3):
                    emit_strip(i)
                if not VX:
                    emit_vtransp(0)
                    emit_vtransp(4)
                emit_proj([2, 3])
                if VX:
                    emit_vtransp_xbar()
                else:
                    emit_vtransp(8)
                    emit_vtransp(12)
                for i in (0, 1, 2, 3):
                    emit_strip_pt(i)
                if not BDEF:
                    emit_band(0)
                for g in (1, 2, 3):
                    prev = None
                    for idx, i in enumerate(range(4 * g, 4 * g + 4)):
                        emit_strip(i)
                        if prev is not None:
                            emit_strip_pt(prev)
                        prev = i
                        if BDEF and idx == 1:
                            emit_band(g - 1)
                    emit_strip_pt(prev)
                    if not BDEF:
                        emit_band(g)
                if BDEF:
                    emit_band(3)

    nc.compile()
    return nc


def _get_nc():
    key = tuple(os.environ.get(k, d) for k, d in (
        ("CBP", "0"), ("PTX", "99"), ("VX", "0"), ("PTW", "1024"),
        ("SCB", "6"), ("PBUF", "3"), ("AUXB", "2"), ("OTB", "2"),
        ("BDEF", "0"), ("CBS", "0")))
    if key not in _NC_CACHE:
        _NC_CACHE[key] = _build()
    return _NC_CACHE[key]


def make_in_maps(x, Wq, bq, Wk, bk, Wv, bv):
    x = np.asarray(x, np.float32)
    xt = np.ascontiguousarray(x.transpose(0, 2, 1))  # [B, D, S]
    Ws = [np.ascontiguousarray(np.asarray(w, np.float32)) for w in (Wq, Wk, Wv)]
    bs = [np.ascontiguousarray(np.asarray(b, np.float32).reshape(H, 1)) for b in (bq, bk, bv)]
    bs[0] = np.ascontiguousarray(bs[0] * 32.0)
    in_maps = []
    for bi in range(B):
        m = {"xT": xt[bi]}
        for p, n in enumerate("qkv"):
            m[f"W{n}"] = Ws[p]
            m[f"b{n}"] = bs[p]
        in_maps.append(m)
    return in_maps


def kernel(x, Wq, bq, Wk, bk, Wv, bv):
    nc = _get_nc()
    in_maps = make_in_maps(x, Wq, bq, Wk, bk, Wv, bv)
    res = run_bass_kernel_spmd(nc, in_maps, list(range(B)))
    outs = []
    for b in range(B):
        oT = res.results[b]["outT"]                      # [H, S]
        sums = res.results[b]["sums"]                    # [128, NS*4]
        s = sums.reshape(128, NS, 4).sum(axis=-1)        # [128, NS]
        s_flat = s.T.reshape(S)                          # s = 128*i + p
        outs.append((oT / s_flat[None, :]).T)
    return np.stack(outs).astype(np.float32)
